# revision 36
# baseline (speedup 1.0000x reference)
"""N-ary TreeLSTM (gnn_message_passing) on 8 TRN2 NeuronCores — v3.

Strategy: data-parallel over batch B=8, one example per core, lean
non-blind recurrent step, one 8-rank AllGather per step.

  * Non-blind step: wait for the AllGather of the previous step's tails,
    then compute y = h@W once (no blind+correction recompute).
  * Gate algebra via scatter/gather-commute identities; o, u and the
    x-projections are loop-invariant and precomputed.
  * The serial post-gate elementwise chain is pipelined in column halves
    (vector/scalar op cost is free-dim-bound), and the hT PSUM->SBUF cast
    is pipelined per k-tile into the y matmuls.
  * The AllGather flight is filled with next-state blend/gate-pre PSUM
    accumulation plus tuned filler matmuls so the PE HAM clock never
    drops to 1.2 GHz.
  * masked_scatter state update as PSUM blend h' = P1^T h_full + Dk^T h
    + P2^T stack with host-built per-core routing matrices; T sized from
    the actual lookback (seed data: 10).
  * Weights and x host-converted to bf16, x pre-transposed.

TensorEngine operands bf16 (fp32 PSUM accumulate); gates in fp32.
"""

import numpy as np
import ml_dtypes

BF16 = ml_dtypes.bfloat16
B, S, H, E, V, NSTEPS = 8, 128, 512, 512, 32000, 8
KT = H // 128   # contraction tiles for K=512
PR = 32         # final-output rows that need the cross-core patch
HH = H // 2     # column half for the elementwise pipeline
FILL = 7        # keep-warm chain links per AllGather window

_last_run = None


def _one_hot_rows(idx):
    m = np.zeros((S, S), np.float32)
    m[np.arange(S), idx] = 1.0
    return m


def _host_prep(inputs):
    tree = np.asarray(inputs["tree_ids"])        # [B, NSTEPS, 3, S]
    input_ids = np.asarray(inputs["input_ids"])  # [B, S]
    emb = np.asarray(inputs["emb"], np.float32)

    # masked_scatter lookback -> T (rows shipped per core per step)
    T = 10
    routing = []
    for t in range(NSTEPS):
        idx_d = tree[:, t, 0, :]
        mask = idx_d != 0
        flat = mask.reshape(-1)
        r_src = (np.cumsum(flat) - flat).reshape(B, S)
        for b in range(B):
            tr = np.nonzero(mask[b])[0]
            if tr.size:
                T = max(T, int(np.max(b * S - r_src[b, tr])) + 1)
        routing.append((mask, r_src))
    assert B * T <= S, f"stack rows {B * T} exceed {S}"

    need_comm = [False] * NSTEPS
    core_mats = [[] for _ in range(B)]  # [128, 9*128] bf16 per (core, step)
    core_cnts = [[] for _ in range(B)]  # [2, 128] bf16 per (core, step)
    pr_last = 1
    for t in range(NSTEPS):
        mask, r_src = routing[t]
        for b in range(B):
            Ar = _one_hot_rows(tree[b, t, 1])
            Al = _one_hot_rows(tree[b, t, 2])
            Ad = _one_hot_rows(tree[b, t, 0])
            cnt_r = Ar.sum(axis=0, dtype=np.float32)
            cnt_l = Al.sum(axis=0, dtype=np.float32)
            P1 = np.zeros((S, S), np.float32)
            Dk = np.diag((~mask[b]).astype(np.float32))
            P2 = np.zeros((S, S), np.float32)  # rows 0:B*T used
            for s in range(S):
                if not mask[b, s]:
                    continue
                src = int(r_src[b, s])
                if src >= b * S:
                    P1[src - b * S, s] = 1.0
                else:
                    assert b > 0
                    q = src - ((b - 1) * S + (S - T))
                    assert 0 <= q < T
                    P2[(b - 1) * T + q, s] = 1.0
                    need_comm[t] = True
                    if t == NSTEPS - 1:
                        pr_last = max(pr_last, s + 1)
            stacked = np.stack(
                [Ar, Al, Ad,
                 np.ascontiguousarray(Ar.T), np.ascontiguousarray(Al.T),
                 np.ascontiguousarray(Ad.T), P1, Dk, P2], 0)
            core_mats[b].append(np.ascontiguousarray(
                stacked.transpose(1, 0, 2).reshape(128, -1)).astype(BF16))
            core_cnts[b].append(
                np.stack([cnt_r, cnt_l], 0).astype(BF16))
    assert pr_last <= PR, f"final patch rows {pr_last} > PR={PR}"

    x_rows = emb[input_ids]  # [B, S, E]

    # o-gate tails of ALL examples, computed locally on every core:
    # o_stk = sigmoid(x_tails @ W_ioux[:, H:2H]) -- loop-invariant, replaces
    # the startup AllGather of o tails.  Ship x tails transposed + k-tiled.
    xtails = x_rows[:, S - T:S, :].reshape(B * T, E)  # [NS, E]
    xtT = np.ascontiguousarray(
        np.concatenate([xtails.T[k * 128:(k + 1) * 128, :]
                        for k in range(KT)], 1)).astype(BF16)  # [128, KT*NS]
    return T, need_comm, core_mats, core_cnts, x_rows, xtT


def _ktile(w):
    """[512, N] -> [128, KT*N] with block k = w[k*128:(k+1)*128, :]."""
    return np.ascontiguousarray(
        np.concatenate([w[k * 128:(k + 1) * 128, :] for k in range(KT)], 1))


def _build_program(T):
    import concourse.bacc as bacc
    import concourse.tile as tile
    import concourse.mybir as mybir
    from contextlib import ExitStack

    dt = mybir.dt
    f32 = dt.float32
    bf16 = dt.bfloat16
    AF = mybir.ActivationFunctionType
    G8 = [list(range(B))]
    NS = B * T  # stack rows

    nc = bacc.Bacc("TRN2", target_bir_lowering=False, debug=False,
                   enable_asserts=False, num_devices=B)

    # ---------------- I/O ----------------
    xT_in = nc.dram_tensor("xT", [128, KT * 128], bf16, kind="ExternalInput")
    xtT_in = nc.dram_tensor("xtT", [128, KT * NS], bf16,
                            kind="ExternalInput")
    W_NAMES = ("W01", "W23", "Wr1", "Wl1")
    w_ins = {n: nc.dram_tensor(n, [128, KT * H], bf16, kind="ExternalInput")
             for n in W_NAMES}
    wfx_in = nc.dram_tensor("Wfx", [128, KT * H], bf16, kind="ExternalInput")
    wioux_in = nc.dram_tensor("Wioux", [128, KT * 3 * H], bf16,
                              kind="ExternalInput")
    bias2_in = nc.dram_tensor("bias2", [2, H], bf16, kind="ExternalInput")
    bf4_in = nc.dram_tensor("bf4", [1, H], bf16, kind="ExternalInput")
    ident_in = nc.dram_tensor("ident", [128, 128], bf16, kind="ExternalInput")
    mats_in = [nc.dram_tensor(f"mats{t}", [128, 9 * 128], bf16,
                              kind="ExternalInput") for t in range(NSTEPS)]
    cnts_in = [nc.dram_tensor(f"cnts{t}", [2, 128], bf16,
                              kind="ExternalInput") for t in range(NSTEPS)]
    out_h = nc.dram_tensor("out_h", [S, H], f32, kind="ExternalOutput")

    CL = (slice(0, HH), slice(HH, H))  # column halves

    with tile.TileContext(nc) as tc:
        with ExitStack() as ctx:
            cpool = ctx.enter_context(tc.tile_pool(name="consts", bufs=1))
            ppool = ctx.enter_context(
                tc.tile_pool(name="psum", bufs=1, space="PSUM"))
            wpool = ctx.enter_context(tc.tile_pool(name="work", bufs=2))
            spool = ctx.enter_context(tc.tile_pool(name="state", bufs=2))
            mpool = ctx.enter_context(tc.tile_pool(name="mats", bufs=3))
            dpool = ctx.enter_context(
                tc.tile_pool(name="dram", bufs=2, space="DRAM"))

            def psum(tag):
                return ppool.tile([S, H], f32, name="ps_" + tag, tag=tag)

            ones_row = cpool.tile([1, 128], bf16, name="ones", tag="ones")
            nc.vector.memset(ones_row, 1.0)

            # ---------------- constants / weights ----------------
            # Spread the ~5MB of startup loads over several engine DMA queues
            # so the precompute's deps (xT, wioux) are not stuck behind the
            # step-1 weights on one queue.
            ident = cpool.tile([128, 128], bf16, name="ident", tag="ident")
            nc.scalar.dma_start(out=ident, in_=ident_in[:, :])
            xT = cpool.tile([128, KT * 128], bf16, name="xT", tag="xT")
            nc.sync.dma_start(out=xT, in_=xT_in[:, :])
            wioux = cpool.tile([128, KT * 3 * H], bf16, name="wioux",
                               tag="wioux")
            nc.sync.dma_start(out=wioux, in_=wioux_in[:, :])
            wfx = cpool.tile([128, KT * H], bf16, name="wfx", tag="wfx")
            nc.gpsimd.dma_start(out=wfx, in_=wfx_in[:, :])
            xtT = cpool.tile([128, KT * NS], bf16, name="xtT", tag="xtT")
            nc.gpsimd.dma_start(out=xtT, in_=xtT_in[:, :])
            w_sb = {}
            for i, n in enumerate(W_NAMES):
                w = cpool.tile([128, KT * H], bf16, name=f"w_{n}",
                               tag=f"w_{n}")
                nc.gpsimd.dma_start(out=w, in_=w_ins[n][:, :])
                w_sb[n] = w
            bias2 = cpool.tile([2, H], bf16, name="bias2", tag="bias2")
            nc.scalar.dma_start(out=bias2, in_=bias2_in[:, :])
            bf4 = cpool.tile([1, H], bf16, name="bf4", tag="bf4")
            nc.scalar.dma_start(out=bf4, in_=bf4_in[:, :])

            def load_mats(t, eng=None):
                eng = eng or nc.sync
                mt = mpool.tile([128, 9 * 128], bf16, name=f"mats{t}",
                                tag="mats")
                eng.dma_start(out=mt, in_=mats_in[t][:, :])
                ct = mpool.tile([2, 128], bf16, name=f"cnts{t}", tag="cnts")
                eng.dma_start(out=ct, in_=cnts_in[t][:, :])
                return mt, ct

            mats_buf = [load_mats(0, nc.scalar), load_mats(1, nc.scalar),
                        load_mats(2, nc.scalar)]

            def M(t, i):
                return mats_buf[t % 3][0][:, i * 128:(i + 1) * 128]

            def CNT(t):
                return mats_buf[t % 3][1]

            # mats slot order
            AR, AL, AD, GRT, GLT, GDT, PP1, DKM, PP2 = range(9)

            # ---- loop-invariant precompute: iou1, o, u, fxb
            iou1 = cpool.tile([S, H], bf16, name="iou1", tag="iou1")
            o_sb = cpool.tile([S, H], f32, name="o_sb", tag="o_sb")
            u_sb = cpool.tile([S, H], f32, name="u_sb", tag="u_sb")
            for i, (dest, func, tag) in enumerate(
                    ((iou1, None, "ya"), (o_sb, AF.Sigmoid, "yb"),
                     (u_sb, AF.Tanh, "ya"))):
                ps = psum(tag)
                for k in range(KT):
                    nc.tensor.matmul(
                        ps, xT[:, k * 128:(k + 1) * 128],
                        wioux[:, k * 3 * H + i * H:k * 3 * H + (i + 1) * H],
                        start=(k == 0), stop=(k == KT - 1))
                if func is None:
                    nc.vector.tensor_copy(dest, ps)
                else:
                    nc.scalar.activation(dest, ps, func)
            fxb = cpool.tile([S, H], bf16, name="fxb", tag="fxb")
            ps_fx = psum("yb")
            for k in range(KT):
                nc.tensor.matmul(ps_fx, xT[:, k * 128:(k + 1) * 128],
                                 wfx[:, k * H:(k + 1) * H],
                                 start=(k == 0), stop=False)
            nc.tensor.matmul(ps_fx, ones_row, bf4, start=False, stop=True)
            nc.vector.tensor_copy(fxb, ps_fx)

            # o tails are loop-invariant AND locally computable on every
            # core from the (shared) x tails of all examples:
            #   o_stk = sigmoid(x_tails @ W_ioux[:, H:2H])
            # This kills the startup AllGather of o tails; the receiver still
            # reconstructs stack_h = o_stk * tanh(stack_c).
            ps_os = psum("f")
            for k in range(KT):
                nc.tensor.matmul(
                    ps_os[0:NS, :], xtT[:, k * NS:(k + 1) * NS],
                    wioux[:, k * 3 * H + H:k * 3 * H + 2 * H],
                    start=(k == 0), stop=(k == KT - 1))
            o_stk = cpool.tile([NS, H], bf16, name="o_stk", tag="o_stk")
            nc.scalar.activation(o_stk, ps_os[0:NS, :], AF.Sigmoid)

            # ---------------- recurrent steps ----------------
            hT_sb = None      # h(t-1)^T bf16 [128, KT*128]
            h_sb = None       # h(t-1) bf16
            cprev_bf = None   # c(t-2) bf16 (Dk rhs for ps_cb)
            ps_hT = ps_h = ps_cb = ps_i = ps_f = None
            ag_prev = None

            def open_gate_pre(t):
                """Open ps_i(t)/ps_f(t) with stack-independent terms."""
                pi = psum("i")
                nc.tensor.matmul(pi, ident, iou1, start=True, stop=False)
                nc.tensor.matmul(pi, CNT(t), bias2, start=False,
                                 stop=(t == 0))
                pf = None
                if t > 0:
                    pf = psum("f")
                    nc.tensor.matmul(pf, M(t, GDT), fxb, start=True,
                                     stop=False)
                return pi, pf

            ps_i, ps_f = open_gate_pre(0)

            for t in range(NSTEPS):
                first = (t == 0)
                last = (t == NSTEPS - 1)

                if not first:
                    # ===== arrival of stack_c(t-1): load, reconstruct
                    # stack_h = o_stk * tanh(stack_c), closers =====
                    stk = spool.tile([NS, H], bf16, name=f"stk{t}",
                                     tag="stk")
                    # split the landing DMA over two queues and the
                    # reconstruct into k-quarters, so the k=0 hT closer,
                    # cast and y matmul start after 128 cols instead of 512
                    nc.sync.dma_start(out=stk[:, CL[0]],
                                      in_=ag_prev[:, 0:HH])
                    nc.scalar.dma_start(out=stk[:, CL[1]],
                                        in_=ag_prev[:, HH:H])
                    P2p = M(t - 1, PP2)
                    nc.tensor.matmul(ps_cb, P2p[0:NS, :], stk[0:NS, :],
                                     start=False, stop=True)
                    tnh = spool.tile([NS, H], bf16, name=f"tnh{t}",
                                     tag="tnh")
                    stk_h = spool.tile([NS, H], bf16, name=f"stkh{t}",
                                       tag="stkh")
                    hT_sb = spool.tile([128, KT * 128], bf16, name=f"hT{t}",
                                       tag="hT")
                    for k in range(KT):
                        sl = slice(k * 128, (k + 1) * 128)
                        nc.scalar.activation(tnh[:, sl], stk[:, sl], AF.Tanh)
                        nc.vector.tensor_mul(stk_h[:, sl], o_stk[:, sl],
                                             tnh[:, sl])
                        nc.tensor.matmul(ps_hT[:, sl], stk_h[0:NS, sl],
                                         P2p[0:NS, :], start=False, stop=True)
                        nc.vector.tensor_copy(hT_sb[:, sl], ps_hT[:, sl])
                    nc.tensor.matmul(ps_h, P2p[0:NS, :], stk_h[0:NS, :],
                                     start=False, stop=True)
                    # mats(t-1) fully consumed; prefetch mats(t+2) into slot
                    if t + 2 < NSTEPS:
                        mats_buf[(t + 2) % 3] = load_mats(t + 2)

                    # ===== chain: y = h(t-1) @ W =====
                    y_sb = {}
                    ps_ys = {}
                    for wi, n in enumerate(W_NAMES):
                        ps_y = psum("ya" if wi % 2 == 0 else "yb")
                        for k in range(KT):
                            nc.tensor.matmul(
                                ps_y, hT_sb[:, k * 128:(k + 1) * 128],
                                w_sb[n][:, k * H:(k + 1) * H],
                                start=(k == 0), stop=(k == KT - 1))
                        ps_ys[n] = ps_y
                        ysb = wpool.tile([S, H], bf16, name=f"y_{n}",
                                         tag=f"y_{n}")
                        y_sb[n] = ysb
                        if n == "W01":
                            nc.vector.tensor_copy(ysb, ps_y)
                        elif n == "W23":
                            nc.vector.tensor_copy(ysb, ps_y)
                            nc.tensor.matmul(ps_f, M(t, GRT), y_sb["W01"],
                                             start=False, stop=False)
                        elif n == "Wr1":
                            nc.tensor.matmul(ps_f, M(t, GLT), y_sb["W23"],
                                             start=False, stop=True)
                            f_sb = wpool.tile([S, H], f32, name="f_sb",
                                              tag="f_sb")
                            nc.scalar.activation(f_sb[:, CL[0]],
                                                 ps_f[:, CL[0]], AF.Sigmoid)
                            nc.scalar.activation(f_sb[:, CL[1]],
                                                 ps_f[:, CL[1]], AF.Sigmoid)
                            nc.vector.tensor_copy(ysb, ps_y)
                            fc = wpool.tile([S, H], bf16, name="fc", tag="fc")
                            nc.vector.tensor_mul(fc[:, CL[0]], f_sb[:, CL[0]],
                                                 ps_cb[:, CL[0]])
                            nc.vector.tensor_mul(fc[:, CL[1]], f_sb[:, CL[1]],
                                                 ps_cb[:, CL[1]])
                        else:
                            nc.vector.tensor_copy(ysb, ps_y)

                    # i-gate closers + scatter of fc (column-half groups)
                    nc.tensor.matmul(ps_i, M(t, AR), y_sb["Wr1"],
                                     start=False, stop=False)
                    nc.tensor.matmul(ps_i, M(t, AL), y_sb["Wl1"],
                                     start=False, stop=True)
                    ps_c = psum("c")
                    nc.tensor.matmul(ps_c[:, CL[0]], M(t, AD), fc[:, CL[0]],
                                     start=True, stop=True)
                    nc.tensor.matmul(ps_c[:, CL[1]], M(t, AD), fc[:, CL[1]],
                                     start=True, stop=True)
                    i_sb = wpool.tile([S, H], f32, name="i_sb", tag="i_sb")
                    iu = wpool.tile([S, H], bf16, name="iu", tag="iu")
                    c_bf = wpool.tile([S, H], bf16, name="c_bf", tag="c_bf")
                    # quarter-split the sigmoid so the tail mul/add (and
                    # hence the AllGather payload ship) fire earlier
                    for k in range(KT):
                        qs = slice(k * 128, (k + 1) * 128)
                        nc.scalar.activation(i_sb[:, qs], ps_i[:, qs],
                                             AF.Sigmoid)
                    TL = slice(S - 32, S)
                    nc.vector.tensor_mul(iu[TL, :], i_sb[TL, :], u_sb[TL, :])
                    nc.vector.tensor_add(c_bf[TL, :], ps_c[TL, :], iu[TL, :])
                    for cs in CL:
                        nc.vector.tensor_mul(iu[0:S - 32, cs],
                                             i_sb[0:S - 32, cs],
                                             u_sb[0:S - 32, cs])
                        nc.vector.tensor_add(c_bf[0:S - 32, cs],
                                             ps_c[0:S - 32, cs],
                                             iu[0:S - 32, cs])
                else:
                    # step 0: h=c=0; c_full = i*u (tail rows first: see above)
                    i_sb = wpool.tile([S, H], f32, name="i_sb", tag="i_sb")
                    c_bf = wpool.tile([S, H], bf16, name="c_bf", tag="c_bf")
                    for cs in CL:
                        nc.scalar.activation(i_sb[:, cs], ps_i[:, cs],
                                             AF.Sigmoid)
                    TL = slice(S - 32, S)
                    nc.vector.tensor_mul(c_bf[TL, :], i_sb[TL, :], u_sb[TL, :])
                    for cs in CL:
                        nc.vector.tensor_mul(c_bf[0:S - 32, cs],
                                             i_sb[0:S - 32, cs],
                                             u_sb[0:S - 32, cs])
                tanh_c = wpool.tile([S, H], f32, name="tanh_c", tag="tanh_c")
                h_full = wpool.tile([S, H], bf16, name="h_full",
                                    tag="h_full")
                if last:
                    # h tail is the final AG payload: compute the tail rows
                    # first so the trigger fires early, then the rest.
                    TL = slice(S - 32, S)
                    nc.scalar.activation(tanh_c[TL, :], c_bf[TL, :], AF.Tanh)
                    nc.vector.tensor_mul(h_full[TL, :], o_sb[TL, :],
                                         tanh_c[TL, :])
                    for cs in CL:
                        nc.scalar.activation(tanh_c[0:S - 32, cs],
                                             c_bf[0:S - 32, cs], AF.Tanh)
                    for cs in CL:
                        nc.vector.tensor_mul(h_full[0:S - 32, cs],
                                             o_sb[0:S - 32, cs],
                                             tanh_c[0:S - 32, cs])

                # ===== ship tails + trigger the AllGather =====
                # steps 0..6 ship the c tail (h is reconstructed on the
                # receiver); the final step ships h for the output patch.
                ag_in = dpool.tile([T, H], bf16, name=f"agin{t}", tag="agin")
                if not last:
                    nc.sync.dma_start(out=ag_in, in_=c_bf[S - T:S, :])
                else:
                    nc.sync.dma_start(out=ag_in, in_=h_full[S - T:S, :])
                ag_out = dpool.tile([NS, H], bf16, name=f"agout{t}",
                                    tag="agout", addr_space="Shared")
                nc.gpsimd.collective_compute(
                    "AllGather", mybir.AluOpType.bypass, replica_groups=G8,
                    ins=[ag_in.opt()], outs=[ag_out.opt()])
                ag_prev = ag_out

                # ===== collective flight: bookkeeping, local h_full,
                # next-state blends, gate pre-terms, keep-warm =====
                if not first and not last:
                    cprev_bf = spool.tile([S, H], bf16, name=f"cpb{t}",
                                          tag="cpb")
                    nc.vector.tensor_copy(cprev_bf, ps_cb)
                if not last:
                    for cs in CL:
                        nc.scalar.activation(tanh_c[:, cs], c_bf[:, cs],
                                             AF.Tanh)
                    for cs in CL:
                        nc.vector.tensor_mul(h_full[:, cs], o_sb[:, cs],
                                             tanh_c[:, cs])
                if not first:
                    h_sb = spool.tile([S, H], bf16, name=f"h{t}", tag="h")
                    nc.vector.tensor_copy(h_sb, ps_h)

                if not last:
                    # c blend + gate pre-terms first: they do not depend on
                    # h_full, so the PE stays busy right after the trigger
                    ps_cb = psum("cb")
                    nc.tensor.matmul(ps_cb, M(t, PP1), c_bf, start=True,
                                     stop=False)
                    if not first:
                        nc.tensor.matmul(ps_cb, M(t, DKM), cprev_bf,
                                         start=False, stop=False)
                    ps_i, ps_f = open_gate_pre(t + 1)
                    ps_h = psum("h")
                    nc.tensor.matmul(ps_h, M(t, PP1), h_full, start=True,
                                     stop=False)
                    if not first:
                        nc.tensor.matmul(ps_h, M(t, DKM), h_sb, start=False,
                                         stop=False)
                    ps_hT = psum("hT")
                    for k in range(KT):
                        sl = slice(k * 128, (k + 1) * 128)
                        nc.tensor.matmul(ps_hT[:, sl], h_full[:, sl],
                                         M(t, PP1), start=True, stop=False)
                        if not first:
                            nc.tensor.matmul(ps_hT[:, sl], h_sb[:, sl],
                                             M(t, DKM), start=False,
                                             stop=False)
                    # keep-warm dependency chain: one short matmul every
                    # ~0.7us through the AllGather flight so the HAM clock
                    # never drops to 1.2 GHz.  Each link's matmul waits on a
                    # scalar-engine copy of the previous link's PSUM, so the
                    # chain paces itself in real time at negligible PE cost.
                    # Anchored on c_bf (just before the collective fires)
                    # and sized to finish before the stack arrives so it
                    # never delays the closers.
                    ps_w = psum("c")
                    kw = c_bf
                    for li in range(FILL):
                        nc.tensor.matmul(ps_w[:, 0:128], ident, kw[:, 0:128],
                                         start=True, stop=True)
                        kw = wpool.tile([128, 128], bf16, name=f"kw{li}",
                                        tag="kw")
                        nc.scalar.activation(kw, ps_w[:, 0:128], AF.Copy)
                else:
                    # final: closed blend for rows >= PR, patch rows < PR
                    ps_h = psum("h")
                    nc.tensor.matmul(ps_h, M(t, PP1), h_full, start=True,
                                     stop=False)
                    nc.tensor.matmul(ps_h, M(t, DKM), h_sb, start=False,
                                     stop=True)
                    h_fin = wpool.tile([S, H], f32, name="h_fin", tag="h_fin")
                    nc.vector.tensor_copy(h_fin, ps_h)
                    nc.scalar.dma_start(out=out_h[PR:S, :],
                                        in_=h_fin[PR:S, :])
                    # patch rows [0:PR] once stack(t) lands (h shipped
                    # directly on the final step)
                    stk = spool.tile([NS, H], bf16, name="stkF", tag="stk")
                    nc.scalar.dma_start(out=stk, in_=ag_prev[:, :])
                    ps_pt = psum("cb")
                    nc.tensor.matmul(ps_pt[0:PR, :], M(t, PP1)[:, 0:PR],
                                     h_full, start=True, stop=False)
                    nc.tensor.matmul(ps_pt[0:PR, :], M(t, DKM)[:, 0:PR],
                                     h_sb, start=False, stop=False)
                    nc.tensor.matmul(ps_pt[0:PR, :], M(t, PP2)[0:NS, 0:PR],
                                     stk[0:NS, :], start=False, stop=True)
                    h_pat = wpool.tile([S, H], f32, name="h_pat", tag="h_fin")
                    nc.vector.tensor_copy(h_pat[0:PR, :], ps_pt[0:PR, :])
                    nc.scalar.dma_start(out=out_h[0:PR, :],
                                        in_=h_pat[0:PR, :])

    # Register a prelude barrier AllGather (inserted at compile time right
    # after the gpsimd preamble, bypassing the Tile scheduler).  It pays the
    # one-time collective-stack setup while the weight DMAs + precompute run,
    # so the first real AllGather doesn't eat the ~30us warm-up.
    nc._bir_kernel_barrier_sem_replica_groups.extend(set(g) for g in G8)

    nc.compile()
    return nc


def kernel(**inputs):
    T, need_comm, core_mats, core_cnts, x_rows, xtT = _host_prep(inputs)

    nc = _build_program(T)

    f = lambda k: np.asarray(inputs[k], np.float32)
    shared = {
        "W01": _ktile(f("W_fh0") + f("W_fh1")).astype(BF16),
        "W23": _ktile(f("W_fh2") + f("W_fh3")).astype(BF16),
        "Wr1": _ktile(np.ascontiguousarray(
            f("W_iouh_r")[:, :H])).astype(BF16),
        "Wl1": _ktile(np.ascontiguousarray(
            f("W_iouh_l")[:, :H])).astype(BF16),
        "Wfx": _ktile(f("W_fx")).astype(BF16),
        "Wioux": _ktile(f("W_ioux")).astype(BF16),
        "bias2": np.stack([f("b_iouh_r")[:H], f("b_iouh_l")[:H]],
                          0).astype(BF16),
        "bf4": (f("b_fh0") + f("b_fh1") + f("b_fh2")
                + f("b_fh3")).reshape(1, H).astype(BF16),
        "ident": np.eye(128, dtype=BF16),
        "xtT": xtT,
    }

    in_maps = []
    for b in range(B):
        m = dict(shared)
        xb = x_rows[b].astype(np.float32)  # [S, E]
        m["xT"] = np.ascontiguousarray(
            np.concatenate([xb[:, k * 128:(k + 1) * 128].T
                            for k in range(KT)], 1)).astype(BF16)
        for t in range(NSTEPS):
            m[f"mats{t}"] = core_mats[b][t]
            m[f"cnts{t}"] = core_cnts[b][t]
        in_maps.append(m)

    from concourse.bass_utils import run_bass_kernel_spmd
    res = run_bass_kernel_spmd(nc, in_maps, core_ids=list(range(B)))
    global _last_run
    _last_run = res
    out = np.stack([res.results[b]["out_h"] for b in range(B)], 0)
    return out.astype(np.float32)

